# revision 1
# baseline (speedup 1.0000x reference)
"""Trainium2 Bass kernel for the contrastive loss problem.

Strategy (8 NeuronCores, SPMD):
  - Core c receives `features` rotated by -1024*c rows, so each core's
    "own" rows are local rows 0..1023 and the single compiled program is
    identical across cores.
  - On device: normalize rows -> z (f32), cast to bf16, transpose via
    TensorE to zT [D, rows].  Each core computes its [1024, 8192] slice
    of E = exp((z @ z.T) / tau) with bf16 matmuls; the ScalarE activation
    computes exp with a fused row-sum (accum_out).  The numerically
    sensitive same-group sums live in the 128x128 diagonal blocks, which
    are recomputed in fp32 and reduced with a host-supplied block-diag
    mask.
  - Host combines per-core row sums:  pos = S - e^(1/tau), neg = T - S,
    loss = mean(log(neg) - log(pos)).
"""

import sys

import numpy as np

sys.path.insert(0, "/opt/trn_rl_repo")

N, D = 8192, 128
NCORES = 8
RPC = N // NCORES  # rows per core (1024)
CHUNKS = N // 128  # 64 row-chunks of 128
ACH = RPC // 128  # own row-chunks per core (8)
TAU = 0.5
EPS = 1e-8

_PROGRAM = None
_COMPILE_PATCHED = False


def _patch_compile():
    """This container's walrus build rejects two instructions that the Tile
    framework emits in its kernel tail: a Drain carrying more than one sem
    wait ("Too many sync wait commands") and the EVENT_SEMAPHORE_RANGE_CLEAR
    ISA instruction ("ISA wrong length").  Rewrite the BIR before walrus sees
    it: split multi-wait Drains into chains of single-wait Drains, and drop
    the range-clear (sems are left dirty, so one NEFF load supports a single
    execution -- kernel() is called once per process, which is our usage)."""
    global _COMPILE_PATCHED
    if _COMPILE_PATCHED:
        return
    import orjson

    import concourse.bass2jax as bass2jax
    import concourse.bass_utils as bass_utils

    orig = bass_utils.compile_bir_kernel

    def patched(bir_json, tmpdir, neff_name="file.neff"):
        bir = orjson.loads(bir_json)
        for fn in bir.get("functions", []):
            for bb in fn.get("blocks", []):
                new_insts = []
                for ins in bb.get("instructions", []):
                    if (
                        ins.get("opcode") == "ISA"
                        and ins.get("isa_opcode") == 176
                    ):
                        continue  # EVENT_SEMAPHORE_RANGE_CLEAR
                    sync = ins.get("sync_info")
                    if sync and len(sync.get("on_wait") or []) > 1:
                        waits = sync["on_wait"]
                        for k, w in enumerate(waits[:-1]):
                            pre = {
                                "engine": ins["engine"],
                                "name": f"{ins['name']}_w{k}",
                                "opcode": "Drain",
                                "ins": [],
                                "outs": [],
                                "sync_info": {"on_update": [], "on_wait": [w]},
                            }
                            if "debug" in ins:
                                pre["debug"] = ins["debug"]
                            new_insts.append(pre)
                        sync["on_wait"] = [waits[-1]]
                    new_insts.append(ins)
                bb["instructions"] = new_insts
        return orig(orjson.dumps(bir), tmpdir, neff_name=neff_name)

    bass_utils.compile_bir_kernel = patched
    bass2jax.compile_bir_kernel = patched
    _COMPILE_PATCHED = True


def _build_program():
    import concourse.bass as bass
    import concourse.mybir as mybir
    import concourse.tile as tile
    from concourse.masks import make_identity

    f32 = mybir.dt.float32
    bf16 = mybir.dt.bfloat16
    AF = mybir.ActivationFunctionType
    AX = mybir.AxisListType
    OP = mybir.AluOpType

    nc = bass.Bass("TRN2", target_bir_lowering=False, debug=False)

    feat = nc.dram_tensor("feat", [N, D], f32, kind="ExternalInput")
    maskd = nc.dram_tensor("mask", [128, 128], f32, kind="ExternalInput")
    t_out = nc.dram_tensor("t_out", [RPC], f32, kind="ExternalOutput")
    tdb_out = nc.dram_tensor("tdb_out", [RPC], f32, kind="ExternalOutput")
    tdf_out = nc.dram_tensor("tdf_out", [RPC], f32, kind="ExternalOutput")
    s_out = nc.dram_tensor("s_out", [RPC], f32, kind="ExternalOutput")

    # DRAM view: row = k*128 + p  ->  [p, k, d]
    feat_r = feat.ap().rearrange("(k p) d -> p k d", p=128)

    G8 = 8  # chunks per DMA/transform group

    with tile.TileContext(nc) as tc:
        with (
            tc.tile_pool(name="singles", bufs=1) as singles,
            tc.tile_pool(name="fload", bufs=3) as fload,
            tc.tile_pool(name="zstage", bufs=2) as zstage,
            tc.tile_pool(name="scratch", bufs=2) as scratch,
            tc.tile_pool(name="eslab", bufs=3) as eslab,
            tc.tile_pool(name="ptr", bufs=2, space="PSUM") as ptr,
            tc.tile_pool(name="ptr32", bufs=2, space="PSUM") as ptr32,
            tc.tile_pool(name="pmain", bufs=2, space="PSUM") as pmain,
        ):
            # ---- constants / persistent buffers ----
            idn_bf = singles.tile([128, 128], bf16)
            make_identity(nc, idn_bf[:])
            idn_32 = singles.tile([128, 128], f32)
            make_identity(nc, idn_32[:])
            mask_sb = singles.tile([128, 128], f32)
            nc.sync.dma_start(mask_sb[:], maskd.ap())

            zT_bf = singles.tile([128, N], bf16)  # [d, local rows]
            zT_32 = singles.tile([128, RPC], f32)  # own chunks, fp32
            ss = singles.tile([128, CHUNKS], f32)
            nrm = singles.tile([128, CHUNKS], f32)
            rcp = singles.tile([128, CHUNKS], f32)
            tacc = singles.tile([128, ACH * 8], f32)
            t_sb = singles.tile([128, ACH], f32)
            tdb_sb = singles.tile([128, ACH], f32)
            tdf_sb = singles.tile([128, ACH], f32)
            s_sb = singles.tile([128, ACH], f32)
            ediag = singles.tile([128, ACH, 128], f32)

            # ---- phase 1: load, normalize, transpose ----
            for g in range(CHUNKS // G8):
                g0 = g * G8
                Fg = fload.tile([128, G8, 128], f32)
                nc.sync.dma_start(Fg[:], feat_r[:, g0 : g0 + G8, :])

                sq = scratch.tile([128, G8, 128], f32, tag="sq")
                nc.vector.tensor_mul(sq[:], Fg[:], Fg[:])
                nc.vector.reduce_sum(
                    out=ss[:, g0 : g0 + G8], in_=sq[:], axis=AX.X
                )
                nc.scalar.sqrt(nrm[:, g0 : g0 + G8], ss[:, g0 : g0 + G8])
                nc.vector.tensor_scalar_max(
                    nrm[:, g0 : g0 + G8], nrm[:, g0 : g0 + G8], EPS
                )
                nc.vector.reciprocal(rcp[:, g0 : g0 + G8], nrm[:, g0 : g0 + G8])

                z8 = zstage.tile([128, G8, 128], bf16, tag="zbf")
                for i in range(G8):
                    nc.vector.tensor_scalar_mul(
                        z8[:, i, :], Fg[:, i, :], rcp[:, g0 + i : g0 + i + 1]
                    )
                trp = ptr.tile([128, G8, 128], bf16)
                for i in range(G8):
                    nc.tensor.transpose(trp[:, i, :], z8[:, i, :], idn_bf[:])
                nc.vector.tensor_copy(
                    zT_bf[:, g0 * 128 : (g0 + G8) * 128],
                    trp[:].rearrange("p a b -> p (a b)"),
                )

                if g == 0:
                    # fp32 z for the own (diagonal) chunks
                    z832 = zstage.tile([128, G8, 128], f32, tag="z32")
                    for i in range(G8):
                        nc.vector.tensor_scalar_mul(
                            z832[:, i, :], Fg[:, i, :], rcp[:, i : i + 1]
                        )
                    for r in range(2):
                        trp32 = ptr32.tile([128, 4, 128], f32, tag="p32")
                        for i in range(4):
                            nc.tensor.transpose(
                                trp32[:, i, :], z832[:, r * 4 + i, :], idn_32[:]
                            )
                        nc.vector.tensor_copy(
                            zT_32[:, r * 512 : (r + 1) * 512],
                            trp32[:].rearrange("p a b -> p (a b)"),
                        )
                    # fp32 diagonal blocks: gram, exp, masked sums
                    for r in range(2):
                        dps = ptr32.tile([128, 4, 128], f32, tag="p32")
                        for i in range(4):
                            A = r * 4 + i
                            nc.tensor.matmul(
                                dps[:, i, :],
                                zT_32[:, A * 128 : (A + 1) * 128],
                                zT_32[:, A * 128 : (A + 1) * 128],
                                start=True,
                                stop=True,
                            )
                        nc.scalar.activation(
                            out=ediag[:, r * 4 : (r + 1) * 4, :].rearrange(
                                "p a b -> p (a b)"
                            ),
                            in_=dps[:].rearrange("p a b -> p (a b)"),
                            func=AF.Exp,
                            scale=2.0,
                        )
                    nc.vector.reduce_sum(out=tdf_sb[:], in_=ediag[:], axis=AX.X)
                    for A in range(ACH):
                        mtmp = scratch.tile([128, 128], f32, tag="sq")
                        nc.vector.tensor_mul(
                            mtmp[:], ediag[:, A, :], mask_sb[:]
                        )
                        nc.vector.reduce_sum(
                            out=s_sb[:, A : A + 1], in_=mtmp[:], axis=AX.X
                        )

            # ---- phase 2: E slabs, fused exp + row sums ----
            for A in range(ACH):
                lhsT = zT_bf[:, A * 128 : (A + 1) * 128]
                for j in range(8):
                    pm = pmain.tile([128, 1024], f32)
                    for m in range(2):
                        c0 = j * 1024 + m * 512
                        nc.tensor.matmul(
                            pm[:, m * 512 : (m + 1) * 512],
                            lhsT,
                            zT_bf[:, c0 : c0 + 512],
                            start=True,
                            stop=True,
                        )
                    es = eslab.tile([128, 1024], bf16)
                    nc.scalar.activation(
                        out=es[:],
                        in_=pm[:],
                        func=AF.Exp,
                        scale=2.0,
                        accum_out=tacc[:, A * 8 + j : A * 8 + j + 1],
                    )
                    if j == 0:
                        # bf16 row-sum of the diagonal block (to be replaced
                        # by the fp32 version on host)
                        nc.vector.reduce_sum(
                            out=tdb_sb[:, A : A + 1],
                            in_=es[:, A * 128 : (A + 1) * 128],
                            axis=AX.X,
                        )

            nc.vector.reduce_sum(
                out=t_sb[:],
                in_=tacc[:].rearrange("p (a j) -> p a j", a=ACH),
                axis=AX.X,
            )

            for sb, dr in (
                (t_sb, t_out),
                (tdb_sb, tdb_out),
                (tdf_sb, tdf_out),
                (s_sb, s_out),
            ):
                nc.sync.dma_start(dr.ap().rearrange("(a p) -> p a", p=128), sb[:])

    return nc


def _get_program():
    global _PROGRAM
    if _PROGRAM is None:
        _PROGRAM = _build_program()
    return _PROGRAM


def _group_ids(num_crops):
    ids = np.repeat(np.arange(num_crops.shape[0], dtype=np.int64), num_crops)
    if ids.shape[0] >= N:
        return ids[:N]
    return np.pad(ids, (0, N - ids.shape[0]), mode="edge")


def _build_mask(num_crops):
    """[128,128] same-group mask, valid when the group pattern repeats
    every 128 rows and no group straddles a 128-row boundary."""
    ids = _group_ids(num_crops)
    pat = ids.reshape(CHUNKS, 128)
    # group-local pattern per chunk must be identical across chunks, and
    # chunks must not share groups
    local = pat - pat[:, :1]
    if not (local == local[0]).all():
        return None
    if (pat[1:, 0] <= pat[:-1, -1]).any():
        return None
    return (local[0][:, None] == local[0][None, :]).astype(np.float32)


def _numpy_fallback(feat, num_crops):
    ids = _group_ids(num_crops)
    nrm = np.maximum(np.sqrt((feat.astype(np.float64) ** 2).sum(-1)), EPS)
    z = feat / nrm[:, None].astype(np.float32)
    T = np.empty(N, np.float64)
    S = np.empty(N, np.float64)
    for r0 in range(0, N, 512):
        E = np.exp((z[r0 : r0 + 512] @ z.T) / TAU).astype(np.float64)
        same = ids[r0 : r0 + 512, None] == ids[None, :]
        T[r0 : r0 + 512] = E.sum(1)
        S[r0 : r0 + 512] = np.where(same, E, 0.0).sum(1)
    pos = S - np.exp(1.0 / TAU)
    neg = T - S
    return np.asarray(np.mean(np.log(neg) - np.log(pos)), dtype=np.float32)


def kernel(features, num_crops):
    feat = np.ascontiguousarray(np.asarray(features, dtype=np.float32))
    ncr = np.asarray(num_crops)
    mask = _build_mask(ncr)
    if mask is None:
        return _numpy_fallback(feat, ncr)

    _patch_compile()
    from concourse.bass_utils import run_bass_kernel_spmd

    nc = _get_program()
    in_maps = [
        {"feat": np.roll(feat, -RPC * c, axis=0).copy(), "mask": mask}
        for c in range(NCORES)
    ]
    res = run_bass_kernel_spmd(nc, in_maps, core_ids=list(range(NCORES)))

    T = np.empty(N, np.float64)
    S = np.empty(N, np.float64)
    for c in range(NCORES):
        r = res.results[c]
        Tc = (
            r["t_out"].astype(np.float64)
            - r["tdb_out"].astype(np.float64)
            + r["tdf_out"].astype(np.float64)
        )
        T[RPC * c : RPC * (c + 1)] = Tc
        S[RPC * c : RPC * (c + 1)] = r["s_out"].astype(np.float64)

    pos = S - np.exp(1.0 / TAU)
    neg = T - S
    loss = np.mean(np.log(neg) - np.log(pos))
    return np.asarray(loss, dtype=np.float32)



# revision 5
# speedup vs baseline: 1.4666x; 1.4666x over previous
"""Trainium2 Bass kernel for the contrastive loss problem.

Strategy (8 NeuronCores, SPMD, symmetric-half algorithm):
  - Host normalizes rows (z = f/max(|f|,eps), fp32) and ships each core a
    pre-transposed slice zT [D=128, 5120] covering its own 1024 rows plus
    the next four 1024-row superblocks (circulant layout: core c gets
    global rows 1024c .. 1024c+5119 mod 8192).  Contiguous DMA, no
    on-device transpose or sqrt.
  - By symmetry E = exp(2 z z^T) is symmetric, so each core computes only
    its [1024, 5120] slice: superblock pairs at circulant distance 1..3
    are computed once (row sums via the ScalarE activation accumulator,
    column sums via a ones-vector TensorE matmul feed the transposed
    pair's row sums); distance 4 is computed by both endpoint cores and
    halved via the activation bias (exp(2x - ln2) = 0.5 exp(2x));
    distance 0 (the diagonal superblock) needs row sums only.
  - The numerically sensitive same-group sums live in the 128x128
    diagonal blocks, recomputed in fp32 and reduced with a host-supplied
    block-diag mask (pos = S - e^2 cancellation needs fp32).
  - Host combines: T[own rows] += rowsums - bf16 diag + fp32 diag;
    T[cols of distance-k superblock] += colsums.  loss =
    mean(log(T-S) - log(S - e^2)).
"""

import sys

import numpy as np

sys.path.insert(0, "/opt/trn_rl_repo")

N, D = 8192, 128
NCORES = 8
RPC = N // NCORES  # rows per core (1024)
KS = 5  # superblocks held per core (k=0..4); k=4 is half-weighted
NR = KS * RPC  # 5120 columns of zT per core
ACH = RPC // 128  # own row-chunks per core (8)
TAU = 0.5
EPS = 1e-8
LN2 = float(np.log(2.0))

_PROGRAM = None
_COMPILE_PATCHED = False


def _patch_compile():
    """This container's walrus build rejects two instructions that the Tile
    framework emits in its kernel tail: a Drain carrying more than one sem
    wait ("Too many sync wait commands") and the EVENT_SEMAPHORE_RANGE_CLEAR
    ISA instruction ("ISA wrong length").  Rewrite the BIR before walrus sees
    it: split multi-wait Drains into chains of single-wait Drains, and drop
    the range-clear (sems are left dirty, so one NEFF load supports a single
    execution -- kernel() is called once per process, which is our usage)."""
    global _COMPILE_PATCHED
    if _COMPILE_PATCHED:
        return
    import orjson

    import concourse.bass2jax as bass2jax
    import concourse.bass_utils as bass_utils

    orig = bass_utils.compile_bir_kernel

    def patched(bir_json, tmpdir, neff_name="file.neff"):
        bir = orjson.loads(bir_json)
        for fn in bir.get("functions", []):
            for bb in fn.get("blocks", []):
                new_insts = []
                for ins in bb.get("instructions", []):
                    if (
                        ins.get("opcode") == "ISA"
                        and ins.get("isa_opcode") == 176
                    ):
                        continue  # EVENT_SEMAPHORE_RANGE_CLEAR
                    sync = ins.get("sync_info")
                    if sync and len(sync.get("on_wait") or []) > 1:
                        waits = sync["on_wait"]
                        for k, w in enumerate(waits[:-1]):
                            pre = {
                                "engine": ins["engine"],
                                "name": f"{ins['name']}_w{k}",
                                "opcode": "Drain",
                                "ins": [],
                                "outs": [],
                                "sync_info": {"on_update": [], "on_wait": [w]},
                            }
                            if "debug" in ins:
                                pre["debug"] = ins["debug"]
                            new_insts.append(pre)
                        sync["on_wait"] = [waits[-1]]
                    new_insts.append(ins)
                bb["instructions"] = new_insts
        return orig(orjson.dumps(bir), tmpdir, neff_name=neff_name)

    bass_utils.compile_bir_kernel = patched
    bass2jax.compile_bir_kernel = patched
    _COMPILE_PATCHED = True


def _build_program():
    import concourse.bass as bass
    import concourse.mybir as mybir
    import concourse.tile as tile

    f32 = mybir.dt.float32
    bf16 = mybir.dt.bfloat16
    AF = mybir.ActivationFunctionType
    AX = mybir.AxisListType

    nc = bass.Bass("TRN2", target_bir_lowering=False, debug=False)

    zt = nc.dram_tensor("zt", [128, NR], f32, kind="ExternalInput")
    maskd = nc.dram_tensor("mask", [128, 128], f32, kind="ExternalInput")
    t_out = nc.dram_tensor("t_out", [RPC], f32, kind="ExternalOutput")
    tdb_out = nc.dram_tensor("tdb_out", [RPC], f32, kind="ExternalOutput")
    tdf_out = nc.dram_tensor("tdf_out", [RPC], f32, kind="ExternalOutput")
    s_out = nc.dram_tensor("s_out", [RPC], f32, kind="ExternalOutput")
    cs_out = nc.dram_tensor("cs_out", [KS - 1, RPC], f32, kind="ExternalOutput")

    with tile.TileContext(nc) as tc:
        with (
            tc.tile_pool(name="singles", bufs=1) as singles,
            tc.tile_pool(name="zstage", bufs=2) as zstage,
            tc.tile_pool(name="scratch", bufs=2) as scratch,
            tc.tile_pool(name="eslab", bufs=3) as eslab,
            tc.tile_pool(name="pmain", bufs=2, space="PSUM") as pmain,
            tc.tile_pool(name="pcs", bufs=1, space="PSUM") as pcs,
            tc.tile_pool(name="pdiag", bufs=1, space="PSUM") as pdiag,
        ):
            # ---- constants / persistent buffers ----
            mask_sb = singles.tile([128, 128], f32)
            nc.sync.dma_start(mask_sb[:], maskd.ap())
            ones_bf = singles.tile([128, 1], bf16)
            nc.vector.memset(ones_bf[:], 1.0)
            nln2 = singles.tile([128, 1], f32)
            nc.vector.memset(nln2[:], -LN2)

            zT_bf = singles.tile([128, NR], bf16)  # [d, rows]
            zT32 = singles.tile([128, RPC], f32)  # own superblock, fp32
            tacc = singles.tile([128, ACH * KS], f32)
            t_sb = singles.tile([128, ACH], f32)
            tdb_sb = singles.tile([128, ACH], f32)
            tdf_sb = singles.tile([128, ACH], f32)
            s_sb = singles.tile([128, ACH], f32)
            ediag = singles.tile([128, ACH, 128], f32)

            # ---- load zT per superblock; cast to bf16 ----
            for k in range(KS):
                if k == 0:
                    nc.sync.dma_start(zT32[:], zt.ap()[:, 0:RPC])
                    nc.vector.tensor_copy(zT_bf[:, 0:RPC], zT32[:])
                else:
                    zs = zstage.tile([128, RPC], f32)
                    nc.sync.dma_start(zs[:], zt.ap()[:, k * RPC : (k + 1) * RPC])
                    nc.vector.tensor_copy(
                        zT_bf[:, k * RPC : (k + 1) * RPC], zs[:]
                    )

            # ---- fp32 diagonal blocks: gram, exp, masked sums ----
            for r in range(2):
                dps = pdiag.tile([128, 4, 128], f32)
                for i in range(4):
                    A = r * 4 + i
                    nc.tensor.matmul(
                        dps[:, i, :],
                        zT32[:, A * 128 : (A + 1) * 128],
                        zT32[:, A * 128 : (A + 1) * 128],
                        start=True,
                        stop=True,
                    )
                nc.scalar.activation(
                    out=ediag[:, r * 4 : (r + 1) * 4, :].rearrange(
                        "p a b -> p (a b)"
                    ),
                    in_=dps[:].rearrange("p a b -> p (a b)"),
                    func=AF.Exp,
                    scale=2.0,
                )
            nc.vector.reduce_sum(out=tdf_sb[:], in_=ediag[:], axis=AX.X)
            for A in range(ACH):
                mtmp = scratch.tile([128, 128], f32, tag="sq")
                nc.vector.tensor_mul(mtmp[:], ediag[:, A, :], mask_sb[:])
                nc.vector.reduce_sum(
                    out=s_sb[:, A : A + 1], in_=mtmp[:], axis=AX.X
                )

            # ---- main slabs: gram, fused exp + row-sum accum, col sums ----
            for k in range(KS):
                if k > 0:
                    cs_lo = pcs.tile([1, 512], f32, tag="lo")
                    cs_hi = pcs.tile([1, 512], f32, tag="hi")
                for A in range(ACH):
                    lhsT = zT_bf[:, A * 128 : (A + 1) * 128]
                    pm = pmain.tile([128, 1024], f32)
                    for m in range(2):
                        c0 = k * RPC + m * 512
                        nc.tensor.matmul(
                            pm[:, m * 512 : (m + 1) * 512],
                            lhsT,
                            zT_bf[:, c0 : c0 + 512],
                            start=True,
                            stop=True,
                        )
                    es = eslab.tile([128, 1024], bf16)
                    nc.scalar.activation(
                        out=es[:],
                        in_=pm[:],
                        func=AF.Exp,
                        scale=2.0,
                        bias=(nln2[:] if k == KS - 1 else 0.0),
                        accum_out=tacc[:, A * KS + k : A * KS + k + 1],
                    )
                    if k == 0:
                        # bf16 row-sum of the diagonal block (replaced by
                        # the fp32 version on host)
                        nc.vector.reduce_sum(
                            out=tdb_sb[:, A : A + 1],
                            in_=es[:, A * 128 : (A + 1) * 128],
                            axis=AX.X,
                        )
                    else:
                        nc.tensor.matmul(
                            cs_lo[:],
                            ones_bf[:],
                            es[:, 0:512],
                            start=(A == 0),
                            stop=(A == ACH - 1),
                            skip_group_check=True,
                        )
                        nc.tensor.matmul(
                            cs_hi[:],
                            ones_bf[:],
                            es[:, 512:1024],
                            start=(A == 0),
                            stop=(A == ACH - 1),
                            skip_group_check=True,
                        )
                if k > 0:
                    cs_sb = scratch.tile([1, 1024], f32, tag="cs")
                    nc.vector.tensor_copy(cs_sb[:, 0:512], cs_lo[:])
                    nc.vector.tensor_copy(cs_sb[:, 512:1024], cs_hi[:])
                    nc.sync.dma_start(cs_out.ap()[k - 1 : k, :], cs_sb[:])

            nc.vector.reduce_sum(
                out=t_sb[:],
                in_=tacc[:].rearrange("p (a k) -> p a k", a=ACH),
                axis=AX.X,
            )

            for sb, dr in (
                (t_sb, t_out),
                (tdb_sb, tdb_out),
                (tdf_sb, tdf_out),
                (s_sb, s_out),
            ):
                nc.sync.dma_start(dr.ap().rearrange("(a p) -> p a", p=128), sb[:])

    return nc


def _get_program():
    global _PROGRAM
    if _PROGRAM is None:
        _PROGRAM = _build_program()
    return _PROGRAM


def _group_ids(num_crops):
    ids = np.repeat(np.arange(num_crops.shape[0], dtype=np.int64), num_crops)
    if ids.shape[0] >= N:
        return ids[:N]
    return np.pad(ids, (0, N - ids.shape[0]), mode="edge")


def _build_mask(num_crops):
    """[128,128] same-group mask, valid when the group pattern repeats
    every 128 rows and no group straddles a 128-row boundary."""
    ids = _group_ids(num_crops)
    pat = ids.reshape(N // 128, 128)
    local = pat - pat[:, :1]
    if not (local == local[0]).all():
        return None
    if (pat[1:, 0] <= pat[:-1, -1]).any():
        return None
    return (local[0][:, None] == local[0][None, :]).astype(np.float32)


def _normalize(feat):
    nrm = np.maximum(np.sqrt((feat.astype(np.float64) ** 2).sum(-1)), EPS)
    return (feat / nrm[:, None]).astype(np.float32)


def _in_maps(feat, mask):
    z = _normalize(feat)
    zT = np.ascontiguousarray(z.T)  # [128, N]
    zT2 = np.concatenate([zT, zT[:, :NR]], axis=1)  # wraparound
    return [
        {
            "zt": np.ascontiguousarray(zT2[:, RPC * c : RPC * c + NR]),
            "mask": mask,
        }
        for c in range(NCORES)
    ]


def _combine(results):
    T = np.zeros(N, np.float64)
    S = np.empty(N, np.float64)
    for c in range(NCORES):
        r = results[c]
        own = slice(RPC * c, RPC * (c + 1))
        T[own] += (
            r["t_out"].astype(np.float64)
            - r["tdb_out"].astype(np.float64)
            + r["tdf_out"].astype(np.float64)
        )
        S[own] = r["s_out"].astype(np.float64)
        cs = r["cs_out"].astype(np.float64)
        for k in range(1, KS):
            cb = (c + k) % NCORES
            T[RPC * cb : RPC * (cb + 1)] += cs[k - 1]
    pos = S - np.exp(1.0 / TAU)
    neg = T - S
    return np.asarray(np.mean(np.log(neg) - np.log(pos)), dtype=np.float32)


def _numpy_fallback(feat, num_crops):
    ids = _group_ids(num_crops)
    z = _normalize(feat)
    T = np.empty(N, np.float64)
    S = np.empty(N, np.float64)
    for r0 in range(0, N, 512):
        E = np.exp((z[r0 : r0 + 512] @ z.T) / TAU).astype(np.float64)
        same = ids[r0 : r0 + 512, None] == ids[None, :]
        T[r0 : r0 + 512] = E.sum(1)
        S[r0 : r0 + 512] = np.where(same, E, 0.0).sum(1)
    pos = S - np.exp(1.0 / TAU)
    neg = T - S
    return np.asarray(np.mean(np.log(neg) - np.log(pos)), dtype=np.float32)


def kernel(features, num_crops):
    feat = np.ascontiguousarray(np.asarray(features, dtype=np.float32))
    ncr = np.asarray(num_crops)
    mask = _build_mask(ncr)
    if mask is None:
        return _numpy_fallback(feat, ncr)

    _patch_compile()
    from concourse.bass_utils import run_bass_kernel_spmd

    nc = _get_program()
    in_maps = _in_maps(feat, mask)
    res = run_bass_kernel_spmd(nc, in_maps, core_ids=list(range(NCORES)))
    return _combine(res.results)


# revision 7
# speedup vs baseline: 1.9970x; 1.3617x over previous
"""Trainium2 Bass kernel for the contrastive loss problem.

Strategy (8 NeuronCores, SPMD, symmetric-half algorithm):
  - Host normalizes rows (z = f/max(|f|,eps), fp32) and ships each core a
    pre-transposed slice zT [D=128, 5120] covering its own 1024 rows plus
    the next four 1024-row superblocks (circulant layout: core c gets
    global rows 1024c .. 1024c+5119 mod 8192).  Contiguous DMA, no
    on-device transpose or sqrt.
  - By symmetry E = exp(2 z z^T) is symmetric, so each core computes only
    its [1024, 5120] slice: superblock pairs at circulant distance 1..3
    are computed once (row sums via the ScalarE activation accumulator,
    column sums via a ones-vector TensorE matmul feed the transposed
    pair's row sums); distance 4 is computed by both endpoint cores and
    halved via the activation bias (exp(2x - ln2) = 0.5 exp(2x));
    distance 0 (the diagonal superblock) needs row sums only.
  - The numerically sensitive same-group sums live in the 128x128
    diagonal blocks, recomputed in fp32 and reduced with a host-supplied
    block-diag mask (pos = S - e^2 cancellation needs fp32).
  - Host combines: T[own rows] += rowsums - bf16 diag + fp32 diag;
    T[cols of distance-k superblock] += colsums.  loss =
    mean(log(T-S) - log(S - e^2)).
"""

import sys

import numpy as np

sys.path.insert(0, "/opt/trn_rl_repo")

N, D = 8192, 128
NCORES = 8
RPC = N // NCORES  # rows per core (1024)
KS = 5  # superblocks held per core (k=0..4); k=4 is half-weighted
NR = KS * RPC  # 5120 columns of zT per core
ACH = RPC // 128  # own row-chunks per core (8)
TAU = 0.5
EPS = 1e-8
LN2 = float(np.log(2.0))

_PROGRAM = None
_COMPILE_PATCHED = False


def _patch_compile():
    """This container's walrus build rejects two instructions that the Tile
    framework emits in its kernel tail: a Drain carrying more than one sem
    wait ("Too many sync wait commands") and the EVENT_SEMAPHORE_RANGE_CLEAR
    ISA instruction ("ISA wrong length").  Rewrite the BIR before walrus sees
    it: split multi-wait Drains into chains of single-wait Drains, and drop
    the range-clear (sems are left dirty, so one NEFF load supports a single
    execution -- kernel() is called once per process, which is our usage)."""
    global _COMPILE_PATCHED
    if _COMPILE_PATCHED:
        return
    import orjson

    import concourse.bass2jax as bass2jax
    import concourse.bass_utils as bass_utils

    orig = bass_utils.compile_bir_kernel

    def patched(bir_json, tmpdir, neff_name="file.neff"):
        bir = orjson.loads(bir_json)
        for fn in bir.get("functions", []):
            for bb in fn.get("blocks", []):
                new_insts = []
                for ins in bb.get("instructions", []):
                    if (
                        ins.get("opcode") == "ISA"
                        and ins.get("isa_opcode") == 176
                    ):
                        continue  # EVENT_SEMAPHORE_RANGE_CLEAR
                    sync = ins.get("sync_info")
                    if sync and len(sync.get("on_wait") or []) > 1:
                        waits = sync["on_wait"]
                        for k, w in enumerate(waits[:-1]):
                            pre = {
                                "engine": ins["engine"],
                                "name": f"{ins['name']}_w{k}",
                                "opcode": "Drain",
                                "ins": [],
                                "outs": [],
                                "sync_info": {"on_update": [], "on_wait": [w]},
                            }
                            if "debug" in ins:
                                pre["debug"] = ins["debug"]
                            new_insts.append(pre)
                        sync["on_wait"] = [waits[-1]]
                    new_insts.append(ins)
                bb["instructions"] = new_insts
        return orig(orjson.dumps(bir), tmpdir, neff_name=neff_name)

    bass_utils.compile_bir_kernel = patched
    bass2jax.compile_bir_kernel = patched
    _COMPILE_PATCHED = True


def _build_program():
    import concourse.bass as bass
    import concourse.mybir as mybir
    import concourse.tile as tile

    f32 = mybir.dt.float32
    bf16 = mybir.dt.bfloat16
    AF = mybir.ActivationFunctionType
    AX = mybir.AxisListType

    nc = bass.Bass("TRN2", target_bir_lowering=False, debug=False)

    zt = nc.dram_tensor("zt", [128, NR], f32, kind="ExternalInput")
    maskd = nc.dram_tensor("mask", [128, 128], f32, kind="ExternalInput")
    t_out = nc.dram_tensor("t_out", [RPC], f32, kind="ExternalOutput")
    tdb_out = nc.dram_tensor("tdb_out", [RPC], f32, kind="ExternalOutput")
    tdf_out = nc.dram_tensor("tdf_out", [RPC], f32, kind="ExternalOutput")
    s_out = nc.dram_tensor("s_out", [RPC], f32, kind="ExternalOutput")
    cs_out = nc.dram_tensor("cs_out", [KS - 1, RPC], f32, kind="ExternalOutput")

    with tile.TileContext(nc) as tc:
        with (
            tc.tile_pool(name="singles", bufs=1) as singles,
            tc.tile_pool(name="zstage", bufs=2) as zstage,
            tc.tile_pool(name="scratch", bufs=2) as scratch,
            tc.tile_pool(name="eslab", bufs=4) as eslab,
            tc.tile_pool(name="pmain", bufs=2, space="PSUM") as pmain,
            tc.tile_pool(name="pcs", bufs=1, space="PSUM") as pcs,
            tc.tile_pool(name="pdiag", bufs=1, space="PSUM") as pdiag,
        ):
            # ---- constants / persistent buffers ----
            mask_sb = singles.tile([128, 128], f32)
            nc.sync.dma_start(mask_sb[:], maskd.ap())
            ones_bf = singles.tile([128, 1], bf16)
            nc.vector.memset(ones_bf[:], 1.0)
            nln2 = singles.tile([128, 1], f32)
            nc.vector.memset(nln2[:], -LN2)

            zT_bf = singles.tile([128, NR], bf16)  # [d, rows]
            zT32 = singles.tile([128, RPC], f32)  # own superblock, fp32
            tacc = singles.tile([128, ACH * KS], f32)
            t_sb = singles.tile([128, ACH], f32)
            tdb_sb = singles.tile([128, ACH], f32)
            tdf_sb = singles.tile([128, ACH], f32)
            s_sb = singles.tile([128, ACH], f32)
            ediag = singles.tile([128, ACH, 128], f32)

            # ---- load zT per superblock; cast to bf16 ----
            for k in range(KS):
                if k == 0:
                    nc.sync.dma_start(zT32[:], zt.ap()[:, 0:RPC])
                    nc.vector.tensor_copy(zT_bf[:, 0:RPC], zT32[:])
                else:
                    zs = zstage.tile([128, RPC], f32)
                    nc.sync.dma_start(zs[:], zt.ap()[:, k * RPC : (k + 1) * RPC])
                    nc.vector.tensor_copy(
                        zT_bf[:, k * RPC : (k + 1) * RPC], zs[:]
                    )

            # ---- main slabs: gram, fused exp + row-sum accum, col sums.
            # The csum matmuls are emitted two iterations behind the grams
            # so the in-order PE queue never stalls waiting for the
            # activation that produces es. ----
            def emit_csum(k, A, es, cs_lo, cs_hi):
                nc.tensor.matmul(
                    cs_lo[:],
                    ones_bf[:],
                    es[:, 0:512],
                    start=(A == 0),
                    stop=(A == ACH - 1),
                    skip_group_check=True,
                )
                nc.tensor.matmul(
                    cs_hi[:],
                    ones_bf[:],
                    es[:, 512:1024],
                    start=(A == 0),
                    stop=(A == ACH - 1),
                    skip_group_check=True,
                )

            for k in range(KS):
                if k > 0:
                    cs_lo = pcs.tile([1, 512], f32, tag="lo")
                    cs_hi = pcs.tile([1, 512], f32, tag="hi")
                pend = []
                for A in range(ACH):
                    lhsT = zT_bf[:, A * 128 : (A + 1) * 128]
                    pm = pmain.tile([128, 1024], f32)
                    for m in range(2):
                        c0 = k * RPC + m * 512
                        nc.tensor.matmul(
                            pm[:, m * 512 : (m + 1) * 512],
                            lhsT,
                            zT_bf[:, c0 : c0 + 512],
                            start=True,
                            stop=True,
                        )
                    if k > 0 and len(pend) >= 2:
                        emit_csum(k, *pend.pop(0), cs_lo, cs_hi)
                    es = eslab.tile([128, 1024], bf16)
                    nc.scalar.activation(
                        out=es[:],
                        in_=pm[:],
                        func=AF.Exp,
                        scale=2.0,
                        bias=(nln2[:] if k == KS - 1 else 0.0),
                        accum_out=tacc[:, A * KS + k : A * KS + k + 1],
                    )
                    if k == 0:
                        # bf16 row-sum of the diagonal block (replaced by
                        # the fp32 version on host)
                        nc.vector.reduce_sum(
                            out=tdb_sb[:, A : A + 1],
                            in_=es[:, A * 128 : (A + 1) * 128],
                            axis=AX.X,
                        )
                    else:
                        pend.append((A, es))
                for A, es in pend:
                    emit_csum(k, A, es, cs_lo, cs_hi)
                if k > 0:
                    cs_sb = scratch.tile([1, 1024], f32, tag="cs")
                    nc.vector.tensor_copy(cs_sb[:, 0:512], cs_lo[:])
                    nc.vector.tensor_copy(cs_sb[:, 512:1024], cs_hi[:])
                    nc.sync.dma_start(cs_out.ap()[k - 1 : k, :], cs_sb[:])
                if k == 0:
                    # fp32 diagonal blocks: gram, exp, masked sums (emitted
                    # here so they fill PE/ACT idle while k>0 slabs stream)
                    for r in range(2):
                        dps = pdiag.tile([128, 4, 128], f32)
                        for i in range(4):
                            A = r * 4 + i
                            nc.tensor.matmul(
                                dps[:, i, :],
                                zT32[:, A * 128 : (A + 1) * 128],
                                zT32[:, A * 128 : (A + 1) * 128],
                                start=True,
                                stop=True,
                            )
                        nc.scalar.activation(
                            out=ediag[:, r * 4 : (r + 1) * 4, :].rearrange(
                                "p a b -> p (a b)"
                            ),
                            in_=dps[:].rearrange("p a b -> p (a b)"),
                            func=AF.Exp,
                            scale=2.0,
                        )
                    nc.vector.reduce_sum(
                        out=tdf_sb[:], in_=ediag[:], axis=AX.X
                    )
                    for A in range(ACH):
                        mtmp = scratch.tile([128, 128], f32, tag="sq")
                        nc.vector.tensor_mul(
                            mtmp[:], ediag[:, A, :], mask_sb[:]
                        )
                        nc.vector.reduce_sum(
                            out=s_sb[:, A : A + 1], in_=mtmp[:], axis=AX.X
                        )

            nc.vector.reduce_sum(
                out=t_sb[:],
                in_=tacc[:].rearrange("p (a k) -> p a k", a=ACH),
                axis=AX.X,
            )

            for sb, dr in (
                (t_sb, t_out),
                (tdb_sb, tdb_out),
                (tdf_sb, tdf_out),
                (s_sb, s_out),
            ):
                nc.sync.dma_start(dr.ap().rearrange("(a p) -> p a", p=128), sb[:])

    return nc


def _get_program():
    global _PROGRAM
    if _PROGRAM is None:
        _PROGRAM = _build_program()
    return _PROGRAM


def _group_ids(num_crops):
    ids = np.repeat(np.arange(num_crops.shape[0], dtype=np.int64), num_crops)
    if ids.shape[0] >= N:
        return ids[:N]
    return np.pad(ids, (0, N - ids.shape[0]), mode="edge")


def _build_mask(num_crops):
    """[128,128] same-group mask, valid when the group pattern repeats
    every 128 rows and no group straddles a 128-row boundary."""
    ids = _group_ids(num_crops)
    pat = ids.reshape(N // 128, 128)
    local = pat - pat[:, :1]
    if not (local == local[0]).all():
        return None
    if (pat[1:, 0] <= pat[:-1, -1]).any():
        return None
    return (local[0][:, None] == local[0][None, :]).astype(np.float32)


def _normalize(feat):
    nrm = np.maximum(np.sqrt((feat.astype(np.float64) ** 2).sum(-1)), EPS)
    return (feat / nrm[:, None]).astype(np.float32)


def _in_maps(feat, mask):
    z = _normalize(feat)
    zT = np.ascontiguousarray(z.T)  # [128, N]
    zT2 = np.concatenate([zT, zT[:, :NR]], axis=1)  # wraparound
    return [
        {
            "zt": np.ascontiguousarray(zT2[:, RPC * c : RPC * c + NR]),
            "mask": mask,
        }
        for c in range(NCORES)
    ]


def _combine(results):
    T = np.zeros(N, np.float64)
    S = np.empty(N, np.float64)
    for c in range(NCORES):
        r = results[c]
        own = slice(RPC * c, RPC * (c + 1))
        T[own] += (
            r["t_out"].astype(np.float64)
            - r["tdb_out"].astype(np.float64)
            + r["tdf_out"].astype(np.float64)
        )
        S[own] = r["s_out"].astype(np.float64)
        cs = r["cs_out"].astype(np.float64)
        for k in range(1, KS):
            cb = (c + k) % NCORES
            T[RPC * cb : RPC * (cb + 1)] += cs[k - 1]
    pos = S - np.exp(1.0 / TAU)
    neg = T - S
    return np.asarray(np.mean(np.log(neg) - np.log(pos)), dtype=np.float32)


def _numpy_fallback(feat, num_crops):
    ids = _group_ids(num_crops)
    z = _normalize(feat)
    T = np.empty(N, np.float64)
    S = np.empty(N, np.float64)
    for r0 in range(0, N, 512):
        E = np.exp((z[r0 : r0 + 512] @ z.T) / TAU).astype(np.float64)
        same = ids[r0 : r0 + 512, None] == ids[None, :]
        T[r0 : r0 + 512] = E.sum(1)
        S[r0 : r0 + 512] = np.where(same, E, 0.0).sum(1)
    pos = S - np.exp(1.0 / TAU)
    neg = T - S
    return np.asarray(np.mean(np.log(neg) - np.log(pos)), dtype=np.float32)


def kernel(features, num_crops):
    feat = np.ascontiguousarray(np.asarray(features, dtype=np.float32))
    ncr = np.asarray(num_crops)
    mask = _build_mask(ncr)
    if mask is None:
        return _numpy_fallback(feat, ncr)

    _patch_compile()
    from concourse.bass_utils import run_bass_kernel_spmd

    nc = _get_program()
    in_maps = _in_maps(feat, mask)
    res = run_bass_kernel_spmd(nc, in_maps, core_ids=list(range(NCORES)))
    return _combine(res.results)


# revision 12
# speedup vs baseline: 2.1547x; 1.0790x over previous
"""Trainium2 Bass kernel for the contrastive loss problem.

Strategy (8 NeuronCores, SPMD, symmetric-half algorithm):
  - Host normalizes rows (z = f/max(|f|,eps), fp32) and ships each core a
    pre-transposed slice zT [D=128, 5120] covering its own 1024 rows plus
    the next four 1024-row superblocks (circulant layout: core c gets
    global rows 1024c .. 1024c+5119 mod 8192).  Contiguous DMA, no
    on-device transpose or sqrt.
  - By symmetry E = exp(2 z z^T) is symmetric, so each core computes only
    its [1024, 5120] slice: superblock pairs at circulant distance 1..3
    are computed once (row sums via the ScalarE activation accumulator,
    column sums via a ones-vector TensorE matmul feed the transposed
    pair's row sums); distance 4 is computed by both endpoint cores and
    halved via the activation bias (exp(2x - ln2) = 0.5 exp(2x));
    distance 0 (the diagonal superblock) needs row sums only.
  - The numerically sensitive same-group sums live in the 128x128
    diagonal blocks, recomputed in fp32 and reduced with a host-supplied
    block-diag mask (pos = S - e^2 cancellation needs fp32).
  - Host combines: T[own rows] += rowsums - bf16 diag + fp32 diag;
    T[cols of distance-k superblock] += colsums.  loss =
    mean(log(T-S) - log(S - e^2)).
"""

import sys

import numpy as np

sys.path.insert(0, "/opt/trn_rl_repo")

N, D = 8192, 128
NCORES = 8
RPC = N // NCORES  # rows per core (1024)
KS = 5  # superblocks held per core (k=0..4); k=4 is half-weighted
NR = KS * RPC  # 5120 columns of zT per core
ACH = RPC // 128  # own row-chunks per core (8)
TAU = 0.5
EPS = 1e-8
LN2 = float(np.log(2.0))

_PROGRAM = None
_COMPILE_PATCHED = False


def _patch_compile():
    """This container's walrus build rejects two instructions that the Tile
    framework emits in its kernel tail: a Drain carrying more than one sem
    wait ("Too many sync wait commands") and the EVENT_SEMAPHORE_RANGE_CLEAR
    ISA instruction ("ISA wrong length").  Rewrite the BIR before walrus sees
    it: split multi-wait Drains into chains of single-wait Drains, and drop
    the range-clear (sems are left dirty, so one NEFF load supports a single
    execution -- kernel() is called once per process, which is our usage)."""
    global _COMPILE_PATCHED
    if _COMPILE_PATCHED:
        return
    import orjson

    import concourse.bass2jax as bass2jax
    import concourse.bass_utils as bass_utils

    orig = bass_utils.compile_bir_kernel

    def patched(bir_json, tmpdir, neff_name="file.neff"):
        bir = orjson.loads(bir_json)
        for fn in bir.get("functions", []):
            for bb in fn.get("blocks", []):
                new_insts = []
                for ins in bb.get("instructions", []):
                    if (
                        ins.get("opcode") == "ISA"
                        and ins.get("isa_opcode") == 176
                    ):
                        continue  # EVENT_SEMAPHORE_RANGE_CLEAR
                    sync = ins.get("sync_info")
                    if sync and len(sync.get("on_wait") or []) > 1:
                        waits = sync["on_wait"]
                        for k, w in enumerate(waits[:-1]):
                            pre = {
                                "engine": ins["engine"],
                                "name": f"{ins['name']}_w{k}",
                                "opcode": "Drain",
                                "ins": [],
                                "outs": [],
                                "sync_info": {"on_update": [], "on_wait": [w]},
                            }
                            if "debug" in ins:
                                pre["debug"] = ins["debug"]
                            new_insts.append(pre)
                        sync["on_wait"] = [waits[-1]]
                    new_insts.append(ins)
                bb["instructions"] = new_insts
        return orig(orjson.dumps(bir), tmpdir, neff_name=neff_name)

    bass_utils.compile_bir_kernel = patched
    bass2jax.compile_bir_kernel = patched
    _COMPILE_PATCHED = True


def _build_program():
    import concourse.bass as bass
    import concourse.mybir as mybir
    import concourse.tile as tile

    f32 = mybir.dt.float32
    bf16 = mybir.dt.bfloat16
    AF = mybir.ActivationFunctionType
    AX = mybir.AxisListType

    nc = bass.Bass("TRN2", target_bir_lowering=False, debug=False)

    zt = nc.dram_tensor("zt", [128, NR], f32, kind="ExternalInput")
    maskd = nc.dram_tensor("mask", [128, 128], f32, kind="ExternalInput")
    t_out = nc.dram_tensor("t_out", [RPC], f32, kind="ExternalOutput")
    tdb_out = nc.dram_tensor("tdb_out", [RPC], f32, kind="ExternalOutput")
    tdf_out = nc.dram_tensor("tdf_out", [RPC], f32, kind="ExternalOutput")
    s_out = nc.dram_tensor("s_out", [RPC], f32, kind="ExternalOutput")
    cs_out = nc.dram_tensor("cs_out", [KS, RPC], f32, kind="ExternalOutput")

    with tile.TileContext(nc) as tc:
        with (
            tc.tile_pool(name="singles", bufs=1) as singles,
            tc.tile_pool(name="zstage", bufs=2) as zstage,
            tc.tile_pool(name="scratch", bufs=2) as scratch,
            tc.tile_pool(name="eslab", bufs=4) as eslab,
            tc.tile_pool(name="pmain", bufs=2, space="PSUM") as pmain,
            tc.tile_pool(name="pcs", bufs=1, space="PSUM") as pcs,
            tc.tile_pool(name="pdiag", bufs=1, space="PSUM") as pdiag,
        ):
            # ---- persistent buffers ----
            zT_bf = singles.tile([128, NR], bf16)  # [d, rows]
            zT32 = singles.tile([128, RPC], f32)  # own superblock, fp32
            mask_sb = singles.tile([128, 128], f32)
            ones_bf = singles.tile([128, 1], bf16)
            zero_bf = singles.tile([128, 128], bf16)
            tacc = singles.tile([128, ACH * KS], f32)
            t_sb = singles.tile([128, ACH], f32)
            tdb_sb = singles.tile([128, ACH], f32)
            tdf_sb = singles.tile([128, ACH], f32)
            s_sb = singles.tile([128, ACH], f32)
            ediag = singles.tile([128, ACH, 128], f32)

            # ---- load zT in 512-col pieces; cast to bf16 (k=0 first so
            # compute starts as early as possible) ----
            for h in range(2 * KS):
                lo, hi = h * 512, (h + 1) * 512
                if h < 2:
                    nc.sync.dma_start(zT32[:, lo:hi], zt.ap()[:, lo:hi])
                    nc.vector.tensor_copy(zT_bf[:, lo:hi], zT32[:, lo:hi])
                else:
                    zs = zstage.tile([128, 512], f32)
                    nc.sync.dma_start(zs[:], zt.ap()[:, lo:hi])
                    nc.vector.tensor_copy(zT_bf[:, lo:hi], zs[:])
                if h == 1:
                    nc.sync.dma_start(mask_sb[:], maskd.ap())
                    nc.vector.memset(ones_bf[:], 1.0)
                    nc.vector.memset(zero_bf[:], 0.0)

            # ---- main slabs: gram, fused exp + row-sum accum, col sums.
            # k=0 and k=4 slabs are chunk-triangular: slab A covers
            # superblock columns [A*128, 1024).  k=0 col sums include the
            # diagonal chunk (host subtracts tdb twice); k=4 col sums are
            # strict (the relative-diagonal chunk's transpose contribution
            # comes from the peer core's row accumulator).
            # The csum matmuls are emitted two iterations behind the grams
            # so the in-order PE queue never stalls waiting for the
            # activation that produces es. ----
            def slab_base(k, A):
                # first superblock column covered by slab (k, A)
                return A * 128 if k in (0, KS - 1) else 0

            def csum_g0(k, A):
                # first superblock column entering the col-sum accumulator
                return A * 128 if k == 0 else ((A + 1) * 128 if k == KS - 1 else 0)

            def emit_csum(k, A, es, cs_lo, cs_hi):
                base = slab_base(k, A)
                g0 = csum_g0(k, A)
                lo_last = {0: 3, KS - 1: 2}.get(k, ACH - 1)
                hi_last = {KS - 1: ACH - 2}.get(k, ACH - 1)
                if g0 < 512:
                    nc.tensor.matmul(
                        cs_lo[:, g0:512],
                        ones_bf[:],
                        es[:, g0 - base : 512 - base],
                        start=(A == 0 and k != KS - 1),
                        stop=(A == lo_last),
                        skip_group_check=True,
                    )
                h0 = max(g0, 512)
                if h0 < 1024:
                    nc.tensor.matmul(
                        cs_hi[:, h0 - 512 : 512],
                        ones_bf[:],
                        es[:, h0 - base : 1024 - base],
                        start=(A == 0),
                        stop=(A == hi_last),
                        skip_group_check=True,
                    )

            for k in range(KS):
                cs_lo = pcs.tile([1, 512], f32, tag="lo")
                cs_hi = pcs.tile([1, 512], f32, tag="hi")
                if k == KS - 1:
                    # init the strict col-sum accumulator's first chunk
                    nc.tensor.matmul(
                        cs_lo[:, 0:128],
                        ones_bf[:],
                        zero_bf[:],
                        start=True,
                        stop=False,
                        skip_group_check=True,
                    )
                pend = []
                for A in range(ACH):
                    base = slab_base(k, A)
                    W = RPC - base
                    lhsT = zT_bf[:, A * 128 : (A + 1) * 128]
                    pm = pmain.tile([128, 1024], f32)
                    for m0 in range(0, W, 512):
                        mw = min(512, W - m0)
                        c0 = k * RPC + base + m0
                        nc.tensor.matmul(
                            pm[:, m0 : m0 + mw],
                            lhsT,
                            zT_bf[:, c0 : c0 + mw],
                            start=True,
                            stop=True,
                        )
                    if len(pend) >= 2:
                        emit_csum(k, *pend.pop(0), cs_lo, cs_hi)
                    es = eslab.tile([128, 1024], bf16)
                    nc.scalar.activation(
                        out=es[:, 0:W],
                        in_=pm[:, 0:W],
                        func=AF.Exp,
                        scale=2.0,
                        accum_out=tacc[:, A * KS + k : A * KS + k + 1],
                    )
                    if k == 0:
                        # bf16 row-sum of the diagonal block (replaced by
                        # the fp32 version on host; also counted once in
                        # the col sums, hence subtracted twice there)
                        nc.vector.reduce_sum(
                            out=tdb_sb[:, A : A + 1],
                            in_=es[:, 0:128],
                            axis=AX.X,
                        )
                    pend.append((A, es))
                for A, es in pend:
                    emit_csum(k, A, es, cs_lo, cs_hi)
                cs_sb = scratch.tile([1, 1024], f32, tag="cs")
                nc.vector.tensor_copy(cs_sb[:, 0:512], cs_lo[:])
                nc.vector.tensor_copy(cs_sb[:, 512:1024], cs_hi[:])
                nc.sync.dma_start(cs_out.ap()[k : k + 1, :], cs_sb[:])
                if k == 0:
                    # fp32 diagonal blocks: gram, exp, masked sums (emitted
                    # here so they fill PE/ACT idle while k>0 slabs stream)
                    for r in range(2):
                        dps = pdiag.tile([128, 4, 128], f32)
                        for i in range(4):
                            A = r * 4 + i
                            nc.tensor.matmul(
                                dps[:, i, :],
                                zT32[:, A * 128 : (A + 1) * 128],
                                zT32[:, A * 128 : (A + 1) * 128],
                                start=True,
                                stop=True,
                            )
                        nc.scalar.activation(
                            out=ediag[:, r * 4 : (r + 1) * 4, :].rearrange(
                                "p a b -> p (a b)"
                            ),
                            in_=dps[:].rearrange("p a b -> p (a b)"),
                            func=AF.Exp,
                            scale=2.0,
                        )
                    nc.vector.reduce_sum(
                        out=tdf_sb[:], in_=ediag[:], axis=AX.X
                    )
                    for A in range(ACH):
                        mtmp = scratch.tile([128, 128], f32, tag="sq")
                        nc.vector.tensor_mul(
                            mtmp[:], ediag[:, A, :], mask_sb[:]
                        )
                        nc.vector.reduce_sum(
                            out=s_sb[:, A : A + 1], in_=mtmp[:], axis=AX.X
                        )

            nc.vector.reduce_sum(
                out=t_sb[:],
                in_=tacc[:].rearrange("p (a k) -> p a k", a=ACH),
                axis=AX.X,
            )

            for sb, dr in (
                (t_sb, t_out),
                (tdb_sb, tdb_out),
                (tdf_sb, tdf_out),
                (s_sb, s_out),
            ):
                nc.sync.dma_start(dr.ap().rearrange("(a p) -> p a", p=128), sb[:])

    return nc


def _get_program():
    global _PROGRAM
    if _PROGRAM is None:
        _PROGRAM = _build_program()
    return _PROGRAM


def _group_ids(num_crops):
    ids = np.repeat(np.arange(num_crops.shape[0], dtype=np.int64), num_crops)
    if ids.shape[0] >= N:
        return ids[:N]
    return np.pad(ids, (0, N - ids.shape[0]), mode="edge")


def _build_mask(num_crops):
    """[128,128] same-group mask, valid when the group pattern repeats
    every 128 rows and no group straddles a 128-row boundary."""
    ids = _group_ids(num_crops)
    pat = ids.reshape(N // 128, 128)
    local = pat - pat[:, :1]
    if not (local == local[0]).all():
        return None
    if (pat[1:, 0] <= pat[:-1, -1]).any():
        return None
    return (local[0][:, None] == local[0][None, :]).astype(np.float32)


def _normalize(feat):
    nrm = np.maximum(np.sqrt((feat.astype(np.float64) ** 2).sum(-1)), EPS)
    return (feat / nrm[:, None]).astype(np.float32)


def _in_maps(feat, mask):
    z = _normalize(feat)
    zT = np.ascontiguousarray(z.T)  # [128, N]
    zT2 = np.concatenate([zT, zT[:, :NR]], axis=1)  # wraparound
    return [
        {
            "zt": np.ascontiguousarray(zT2[:, RPC * c : RPC * c + NR]),
            "mask": mask,
        }
        for c in range(NCORES)
    ]


def _combine(results):
    T = np.zeros(N, np.float64)
    S = np.empty(N, np.float64)
    for c in range(NCORES):
        r = results[c]
        own = slice(RPC * c, RPC * (c + 1))
        cs = r["cs_out"].astype(np.float64)
        # cs[0] covers the own-superblock lower triangle; the diagonal
        # chunks appear in both t_out and cs[0], so tdb is removed twice.
        T[own] += (
            r["t_out"].astype(np.float64)
            + cs[0]
            - 2.0 * r["tdb_out"].astype(np.float64)
            + r["tdf_out"].astype(np.float64)
        )
        S[own] = r["s_out"].astype(np.float64)
        for k in range(1, KS):
            cb = (c + k) % NCORES
            T[RPC * cb : RPC * (cb + 1)] += cs[k]
    pos = S - np.exp(1.0 / TAU)
    neg = T - S
    return np.asarray(np.mean(np.log(neg) - np.log(pos)), dtype=np.float32)


def _numpy_fallback(feat, num_crops):
    ids = _group_ids(num_crops)
    z = _normalize(feat)
    T = np.empty(N, np.float64)
    S = np.empty(N, np.float64)
    for r0 in range(0, N, 512):
        E = np.exp((z[r0 : r0 + 512] @ z.T) / TAU).astype(np.float64)
        same = ids[r0 : r0 + 512, None] == ids[None, :]
        T[r0 : r0 + 512] = E.sum(1)
        S[r0 : r0 + 512] = np.where(same, E, 0.0).sum(1)
    pos = S - np.exp(1.0 / TAU)
    neg = T - S
    return np.asarray(np.mean(np.log(neg) - np.log(pos)), dtype=np.float32)


def kernel(features, num_crops):
    feat = np.ascontiguousarray(np.asarray(features, dtype=np.float32))
    ncr = np.asarray(num_crops)
    mask = _build_mask(ncr)
    if mask is None:
        return _numpy_fallback(feat, ncr)

    _patch_compile()
    from concourse.bass_utils import run_bass_kernel_spmd

    nc = _get_program()
    in_maps = _in_maps(feat, mask)
    res = run_bass_kernel_spmd(nc, in_maps, core_ids=list(range(NCORES)))
    return _combine(res.results)


# revision 15
# speedup vs baseline: 2.1594x; 1.0022x over previous
"""Trainium2 Bass kernel for the contrastive loss problem.

Strategy (8 NeuronCores, SPMD, symmetric-half algorithm):
  - Host normalizes rows (z = f/max(|f|,eps), fp32) and ships each core a
    pre-transposed slice zT [D=128, 5120] covering its own 1024 rows plus
    the next four 1024-row superblocks (circulant layout: core c gets
    global rows 1024c .. 1024c+5119 mod 8192).  Contiguous DMA, no
    on-device transpose or sqrt.
  - By symmetry E = exp(2 z z^T) is symmetric, so each core computes only
    its [1024, 5120] slice: superblock pairs at circulant distance 1..3
    are computed once (row sums via the ScalarE activation accumulator,
    column sums via a ones-vector TensorE matmul feed the transposed
    pair's row sums); distance 4 is computed by both endpoint cores and
    halved via the activation bias (exp(2x - ln2) = 0.5 exp(2x));
    distance 0 (the diagonal superblock) needs row sums only.
  - The numerically sensitive same-group sums live in the 128x128
    diagonal blocks, recomputed in fp32 and reduced with a host-supplied
    block-diag mask (pos = S - e^2 cancellation needs fp32).
  - Host combines: T[own rows] += rowsums - bf16 diag + fp32 diag;
    T[cols of distance-k superblock] += colsums.  loss =
    mean(log(T-S) - log(S - e^2)).
"""

import sys

import numpy as np

sys.path.insert(0, "/opt/trn_rl_repo")

N, D = 8192, 128
NCORES = 8
RPC = N // NCORES  # rows per core (1024)
KS = 5  # superblocks held per core (k=0..4); k=4 is half-weighted
NR = KS * RPC  # 5120 columns of zT per core
ACH = RPC // 128  # own row-chunks per core (8)
TAU = 0.5
EPS = 1e-8
LN2 = float(np.log(2.0))

_PROGRAM = None
_COMPILE_PATCHED = False


def _patch_compile():
    """This container's walrus build rejects two instructions that the Tile
    framework emits in its kernel tail: a Drain carrying more than one sem
    wait ("Too many sync wait commands") and the EVENT_SEMAPHORE_RANGE_CLEAR
    ISA instruction ("ISA wrong length").  Rewrite the BIR before walrus sees
    it: split multi-wait Drains into chains of single-wait Drains, and drop
    the range-clear (sems are left dirty, so one NEFF load supports a single
    execution -- kernel() is called once per process, which is our usage)."""
    global _COMPILE_PATCHED
    if _COMPILE_PATCHED:
        return
    import orjson

    import concourse.bass2jax as bass2jax
    import concourse.bass_utils as bass_utils

    orig = bass_utils.compile_bir_kernel

    def patched(bir_json, tmpdir, neff_name="file.neff"):
        bir = orjson.loads(bir_json)
        for fn in bir.get("functions", []):
            for bb in fn.get("blocks", []):
                new_insts = []
                for ins in bb.get("instructions", []):
                    if (
                        ins.get("opcode") == "ISA"
                        and ins.get("isa_opcode") == 176
                    ):
                        continue  # EVENT_SEMAPHORE_RANGE_CLEAR
                    sync = ins.get("sync_info")
                    if sync and len(sync.get("on_wait") or []) > 1:
                        waits = sync["on_wait"]
                        for k, w in enumerate(waits[:-1]):
                            pre = {
                                "engine": ins["engine"],
                                "name": f"{ins['name']}_w{k}",
                                "opcode": "Drain",
                                "ins": [],
                                "outs": [],
                                "sync_info": {"on_update": [], "on_wait": [w]},
                            }
                            if "debug" in ins:
                                pre["debug"] = ins["debug"]
                            new_insts.append(pre)
                        sync["on_wait"] = [waits[-1]]
                    new_insts.append(ins)
                bb["instructions"] = new_insts
        return orig(orjson.dumps(bir), tmpdir, neff_name=neff_name)

    bass_utils.compile_bir_kernel = patched
    bass2jax.compile_bir_kernel = patched
    _COMPILE_PATCHED = True


def _build_program():
    import concourse.bass as bass
    import concourse.mybir as mybir
    import concourse.tile as tile

    f32 = mybir.dt.float32
    bf16 = mybir.dt.bfloat16
    AF = mybir.ActivationFunctionType
    AX = mybir.AxisListType

    nc = bass.Bass("TRN2", target_bir_lowering=False, debug=False)

    zt = nc.dram_tensor("zt", [128, NR], f32, kind="ExternalInput")
    maskd = nc.dram_tensor("mask", [128, 128], f32, kind="ExternalInput")
    t_out = nc.dram_tensor("t_out", [RPC], f32, kind="ExternalOutput")
    tdb_out = nc.dram_tensor("tdb_out", [RPC], f32, kind="ExternalOutput")
    tdf_out = nc.dram_tensor("tdf_out", [RPC], f32, kind="ExternalOutput")
    s_out = nc.dram_tensor("s_out", [RPC], f32, kind="ExternalOutput")
    cs_out = nc.dram_tensor("cs_out", [KS, RPC], f32, kind="ExternalOutput")

    with tile.TileContext(nc) as tc:
        with (
            tc.tile_pool(name="singles", bufs=1) as singles,
            tc.tile_pool(name="zstage", bufs=2) as zstage,
            tc.tile_pool(name="scratch", bufs=2) as scratch,
            tc.tile_pool(name="eslab", bufs=4) as eslab,
            tc.tile_pool(name="pmain", bufs=2, space="PSUM") as pmain,
            tc.tile_pool(name="pcs", bufs=1, space="PSUM") as pcs,
            tc.tile_pool(name="pdiag", bufs=1, space="PSUM") as pdiag,
        ):
            # ---- persistent buffers ----
            zT_bf = singles.tile([128, NR], bf16)  # [d, rows]
            zT32 = singles.tile([128, RPC], f32)  # own superblock, fp32
            mask_sb = singles.tile([128, 128], f32)
            ones_bf = singles.tile([128, 1], bf16)
            zero_bf = singles.tile([128, 128], bf16)
            tacc = singles.tile([128, ACH * KS], f32)
            t_sb = singles.tile([128, ACH], f32)
            tdb_sb = singles.tile([128, ACH], f32)
            tdf_sb = singles.tile([128, ACH], f32)
            s_sb = singles.tile([128, ACH], f32)
            ediag = singles.tile([128, ACH, 128], f32)

            # ---- load zT in 512-col pieces; cast to bf16 (k=0 first so
            # compute starts as early as possible) ----
            for h in range(2 * KS):
                lo, hi = h * 512, (h + 1) * 512
                if h < 2:
                    nc.sync.dma_start(zT32[:, lo:hi], zt.ap()[:, lo:hi])
                    nc.vector.tensor_copy(zT_bf[:, lo:hi], zT32[:, lo:hi])
                else:
                    zs = zstage.tile([128, 512], f32)
                    nc.sync.dma_start(zs[:], zt.ap()[:, lo:hi])
                    nc.vector.tensor_copy(zT_bf[:, lo:hi], zs[:])
                if h == 1:
                    nc.sync.dma_start(mask_sb[:], maskd.ap())
                    nc.vector.memset(ones_bf[:], 1.0)
                    nc.vector.memset(zero_bf[:], 0.0)

            # ---- main slabs: gram, fused exp + row-sum accum, col sums.
            # k=0 and k=4 slabs are chunk-triangular: slab A covers
            # superblock columns [A*128, 1024).  k=0 col sums include the
            # diagonal chunk (host subtracts tdb twice); k=4 col sums are
            # strict (the relative-diagonal chunk's transpose contribution
            # comes from the peer core's row accumulator).
            # The csum matmuls are emitted two iterations behind the grams
            # so the in-order PE queue never stalls waiting for the
            # activation that produces es. ----
            def slab_base(k, A):
                # first superblock column covered by slab (k, A)
                return A * 128 if k in (0, KS - 1) else 0

            def csum_g0(k, A):
                # first superblock column entering the col-sum accumulator
                return A * 128 if k == 0 else ((A + 1) * 128 if k == KS - 1 else 0)

            def emit_csum(k, A, es, cs_lo, cs_hi):
                base = slab_base(k, A)
                g0 = csum_g0(k, A)
                lo_last = {0: 3, KS - 1: 2}.get(k, ACH - 1)
                hi_last = {KS - 1: ACH - 2}.get(k, ACH - 1)
                if g0 < 512:
                    nc.tensor.matmul(
                        cs_lo[:, g0:512],
                        ones_bf[:],
                        es[:, g0 - base : 512 - base],
                        start=(A == 0 and k != KS - 1),
                        stop=(A == lo_last),
                        skip_group_check=True,
                    )
                h0 = max(g0, 512)
                if h0 < 1024:
                    nc.tensor.matmul(
                        cs_hi[:, h0 - 512 : 512],
                        ones_bf[:],
                        es[:, h0 - base : 1024 - base],
                        start=(A == 0),
                        stop=(A == hi_last),
                        skip_group_check=True,
                    )

            for k in range(KS):
                cs_lo = pcs.tile([1, 512], f32, tag="lo")
                cs_hi = pcs.tile([1, 512], f32, tag="hi")
                if k == KS - 1:
                    # init the strict col-sum accumulator's first chunk
                    nc.tensor.matmul(
                        cs_lo[:, 0:128],
                        ones_bf[:],
                        zero_bf[:],
                        start=True,
                        stop=False,
                        skip_group_check=True,
                    )
                pend = []
                for A in range(ACH):
                    base = slab_base(k, A)
                    W = RPC - base
                    lhsT = zT_bf[:, A * 128 : (A + 1) * 128]
                    pm = pmain.tile([128, 1024], f32)
                    for m0 in range(0, W, 512):
                        mw = min(512, W - m0)
                        c0 = k * RPC + base + m0
                        nc.tensor.matmul(
                            pm[:, m0 : m0 + mw],
                            lhsT,
                            zT_bf[:, c0 : c0 + mw],
                            start=True,
                            stop=True,
                        )
                    if len(pend) >= 2:
                        emit_csum(k, *pend.pop(0), cs_lo, cs_hi)
                    es = eslab.tile([128, 1024], bf16)
                    if k in (0, KS - 1) and A >= 4:
                        # small ragged slabs: row-sum on the idle Vector
                        # engine instead of paying READ_ACCUMULATOR
                        nc.scalar.activation(
                            out=es[:, 0:W],
                            in_=pm[:, 0:W],
                            func=AF.Exp,
                            scale=2.0,
                        )
                        nc.vector.reduce_sum(
                            out=tacc[:, A * KS + k : A * KS + k + 1],
                            in_=es[:, 0:W],
                            axis=AX.X,
                        )
                    else:
                        nc.scalar.activation(
                            out=es[:, 0:W],
                            in_=pm[:, 0:W],
                            func=AF.Exp,
                            scale=2.0,
                            accum_out=tacc[:, A * KS + k : A * KS + k + 1],
                        )
                    if k == 0:
                        # bf16 row-sum of the diagonal block (replaced by
                        # the fp32 version on host; also counted once in
                        # the col sums, hence subtracted twice there)
                        nc.vector.reduce_sum(
                            out=tdb_sb[:, A : A + 1],
                            in_=es[:, 0:128],
                            axis=AX.X,
                        )
                    pend.append((A, es))
                for A, es in pend:
                    emit_csum(k, A, es, cs_lo, cs_hi)
                cs_sb = scratch.tile([1, 1024], f32, tag="cs")
                nc.vector.tensor_copy(cs_sb[:, 0:512], cs_lo[:])
                nc.vector.tensor_copy(cs_sb[:, 512:1024], cs_hi[:])
                nc.sync.dma_start(cs_out.ap()[k : k + 1, :], cs_sb[:])
                if k == 0:
                    # fp32 diagonal blocks: gram, exp, masked sums (emitted
                    # here so they fill PE/ACT idle while k>0 slabs stream)
                    dps = pdiag.tile([128, ACH, 128], f32)
                    for A in range(ACH):
                        nc.tensor.matmul(
                            dps[:, A, :],
                            zT32[:, A * 128 : (A + 1) * 128],
                            zT32[:, A * 128 : (A + 1) * 128],
                            start=True,
                            stop=True,
                        )
                    nc.scalar.activation(
                        out=ediag[:].rearrange("p a b -> p (a b)"),
                        in_=dps[:].rearrange("p a b -> p (a b)"),
                        func=AF.Exp,
                        scale=2.0,
                    )
                    nc.vector.reduce_sum(
                        out=tdf_sb[:], in_=ediag[:], axis=AX.X
                    )
                    for A in range(ACH):
                        mtmp = scratch.tile([128, 128], f32, tag="sq")
                        nc.vector.tensor_mul(
                            mtmp[:], ediag[:, A, :], mask_sb[:]
                        )
                        nc.vector.reduce_sum(
                            out=s_sb[:, A : A + 1], in_=mtmp[:], axis=AX.X
                        )
                    for sb, dr in (
                        (tdb_sb, tdb_out),
                        (tdf_sb, tdf_out),
                        (s_sb, s_out),
                    ):
                        nc.sync.dma_start(
                            dr.ap().rearrange("(a p) -> p a", p=128), sb[:]
                        )

            nc.vector.reduce_sum(
                out=t_sb[:],
                in_=tacc[:].rearrange("p (a k) -> p a k", a=ACH),
                axis=AX.X,
            )

            nc.sync.dma_start(
                t_out.ap().rearrange("(a p) -> p a", p=128), t_sb[:]
            )

    return nc


def _get_program():
    global _PROGRAM
    if _PROGRAM is None:
        _PROGRAM = _build_program()
    return _PROGRAM


def _group_ids(num_crops):
    ids = np.repeat(np.arange(num_crops.shape[0], dtype=np.int64), num_crops)
    if ids.shape[0] >= N:
        return ids[:N]
    return np.pad(ids, (0, N - ids.shape[0]), mode="edge")


def _build_mask(num_crops):
    """[128,128] same-group mask, valid when the group pattern repeats
    every 128 rows and no group straddles a 128-row boundary."""
    ids = _group_ids(num_crops)
    pat = ids.reshape(N // 128, 128)
    local = pat - pat[:, :1]
    if not (local == local[0]).all():
        return None
    if (pat[1:, 0] <= pat[:-1, -1]).any():
        return None
    return (local[0][:, None] == local[0][None, :]).astype(np.float32)


def _normalize(feat):
    nrm = np.maximum(np.sqrt((feat.astype(np.float64) ** 2).sum(-1)), EPS)
    return (feat / nrm[:, None]).astype(np.float32)


def _in_maps(feat, mask):
    z = _normalize(feat)
    zT = np.ascontiguousarray(z.T)  # [128, N]
    zT2 = np.concatenate([zT, zT[:, :NR]], axis=1)  # wraparound
    return [
        {
            "zt": np.ascontiguousarray(zT2[:, RPC * c : RPC * c + NR]),
            "mask": mask,
        }
        for c in range(NCORES)
    ]


def _combine(results):
    T = np.zeros(N, np.float64)
    S = np.empty(N, np.float64)
    for c in range(NCORES):
        r = results[c]
        own = slice(RPC * c, RPC * (c + 1))
        cs = r["cs_out"].astype(np.float64)
        # cs[0] covers the own-superblock lower triangle; the diagonal
        # chunks appear in both t_out and cs[0], so tdb is removed twice.
        T[own] += (
            r["t_out"].astype(np.float64)
            + cs[0]
            - 2.0 * r["tdb_out"].astype(np.float64)
            + r["tdf_out"].astype(np.float64)
        )
        S[own] = r["s_out"].astype(np.float64)
        for k in range(1, KS):
            cb = (c + k) % NCORES
            T[RPC * cb : RPC * (cb + 1)] += cs[k]
    pos = S - np.exp(1.0 / TAU)
    neg = T - S
    return np.asarray(np.mean(np.log(neg) - np.log(pos)), dtype=np.float32)


def _numpy_fallback(feat, num_crops):
    ids = _group_ids(num_crops)
    z = _normalize(feat)
    T = np.empty(N, np.float64)
    S = np.empty(N, np.float64)
    for r0 in range(0, N, 512):
        E = np.exp((z[r0 : r0 + 512] @ z.T) / TAU).astype(np.float64)
        same = ids[r0 : r0 + 512, None] == ids[None, :]
        T[r0 : r0 + 512] = E.sum(1)
        S[r0 : r0 + 512] = np.where(same, E, 0.0).sum(1)
    pos = S - np.exp(1.0 / TAU)
    neg = T - S
    return np.asarray(np.mean(np.log(neg) - np.log(pos)), dtype=np.float32)


def kernel(features, num_crops):
    feat = np.ascontiguousarray(np.asarray(features, dtype=np.float32))
    ncr = np.asarray(num_crops)
    mask = _build_mask(ncr)
    if mask is None:
        return _numpy_fallback(feat, ncr)

    _patch_compile()
    from concourse.bass_utils import run_bass_kernel_spmd

    nc = _get_program()
    in_maps = _in_maps(feat, mask)
    res = run_bass_kernel_spmd(nc, in_maps, core_ids=list(range(NCORES)))
    return _combine(res.results)
